# revision 21
# baseline (speedup 1.0000x reference)
"""3x3 VALID conv (NCHW) on 8 Trainium2 NeuronCores, data-parallel on batch.

Contract: kernel(img, filtro) takes the FULL inputs
  img    [32, 128, 56, 56] f32
  filtro [256, 128, 3, 3]  f32
and returns the FULL output [32, 256, 54, 54] f32.

Strategy (per core, batch shard of 4 images):
- Inputs are host-cast to bf16 (rel-err budget 2e-2 >> bf16's ~4e-3 for
  K=1152 fp32-accumulated reductions). bf16 matmuls stream 1 col/cycle
  like fp32r, but their weight loads are emitted as standalone Ldweights
  instructions (FWL, ~53ns) that the PE's 64-deep reorder window hides
  behind in-flight matmuls - fp32r self-loading matmuls instead pay a
  serial ~107ns reload inside every matmul (~46us/rep).
- img in SBUF channels-on-partitions: [ci=128, n, h, w] bf16 (25KB/part).
- w host-packed to [ci, cb, tap, co128] so each (cb, tap) slice
  [128, 128] is one stationary load.
- Schedule: cb-major, then per image a wave of 6 row-groups (9 rows x 54
  = 486 cols = one PSUM bank each), taps outer so one weight load serves
  6 matmuls; 8-bank PSUM rotation lets the next wave start on banks 6,7
  while this wave's banks drain.
- Drain: DVE copies each bank to an SBUF tile; out DMA goes on the ACT
  HWDGE ring (nc.scalar.dma_start) so stores never head-of-line block
  the SP ring that prefetches the next iteration's images.
"""
from contextlib import ExitStack

import numpy as np

BATCH, C_IN, C_OUT, H, K = 32, 128, 256, 56, 3
OH = H - K + 1  # 54
N_CORES = 8
PER = BATCH // N_CORES  # 4
RG = 9          # output rows per matmul group; 9*54=486 <= 512-f32 PSUM bank
NG = OH // RG   # 6

_CACHE = {}
DEDUP = True


def _dedup_ldweights(nc):
    """Remove Ldweights whose stationary AP matches the weights already in
    the PE array (tile legalization emits one per matmul; taps-outer reuses
    each load 6x). Any waits/updates on a removed load move to the next PE
    instruction."""
    removed = 0
    for blk in nc.m.functions[0].blocks:
        insts = list(blk.instructions)
        last_w = None
        drop, pending = [], []
        for i, inst in enumerate(insts):
            if str(getattr(inst, "engine", "")) != "EngineType.PE":
                continue
            if pending and inst.opcode in ("Matmult", "Ldweights"):
                si = inst.sync_info
                waits = list(si.on_wait) if si else []
                ups = list(si.on_update) if si else []
                from concourse import mybir as _mb
                for psi in pending:
                    waits += list(psi.on_wait)
                    ups += list(psi.on_update)
                inst.sync_info = _mb.SyncInfo(on_wait=waits, on_update=ups)
                pending = []
            if inst.opcode == "Ldweights":
                sig = str(inst.ins[0])
                if sig == last_w:
                    si = inst.sync_info
                    if si and (len(si.on_wait) or len(si.on_update)):
                        pending.append(si)
                    drop.append(i)
                else:
                    last_w = sig
            elif inst.opcode == "Matmult":
                if inst.ldweights is not False:
                    last_w = str(inst.ins[1])
            else:
                last_w = None
        assert not pending
        for i in reversed(drop):
            del blk.instructions[i]
        removed += len(drop)
    return removed


def _build(reps=1, drop_out=False, drop_mm=False, flat_rhs=False,
           single_w=False, drain_engine="vector", out_ring="scalar",
           wave_sz=4, in_bufs=2, mm_split=1):
    import concourse.tile as tile
    from concourse import bacc, mybir

    BF = mybir.dt.bfloat16
    F32 = mybir.dt.float32
    first_chunks = 3

    nc = bacc.Bacc(None, target_bir_lowering=False)
    img = nc.declare_dram_parameter("img", [PER, C_IN, H, H], BF,
                                    isOutput=False)
    w = nc.declare_dram_parameter("w", [C_IN, 2 * K * K * 128], BF,
                                  isOutput=False)
    out = nc.declare_dram_parameter("out", [PER, C_OUT, OH, OH], F32,
                                    isOutput=True)

    with tile.TileContext(nc) as tc:
        with ExitStack() as ctx:
            wpool = ctx.enter_context(
                tc.tile_pool(name="wpool", bufs=in_bufs))
            imgpool = ctx.enter_context(
                tc.tile_pool(name="imgpool", bufs=in_bufs))
            psum_pool = ctx.enter_context(
                tc.tile_pool(name="psum", bufs=8, space="PSUM"))
            outp = ctx.enter_context(tc.tile_pool(name="outp", bufs=8))

            def body():
                w_sb = wpool.tile([C_IN, 2 * K * K * 128], BF)
                # cb=0 block first so the first wave's weights land early
                half = K * K * 128
                nc.sync.dma_start(out=w_sb[:, :half], in_=w[:, :half])
                img_sb = imgpool.tile([C_IN, PER, H, H], BF)
                src = img.rearrange("n c h w -> c n h w")
                bounds = [0]
                step = (H + first_chunks - 1) // first_chunks
                while bounds[-1] < H:
                    bounds.append(min(bounds[-1] + step, H))
                for r0, r1 in zip(bounds[:-1], bounds[1:]):
                    nc.sync.dma_start(out=img_sb[:, 0, r0:r1],
                                      in_=src[:, 0, r0:r1])
                for n in range(1, PER):
                    nc.sync.dma_start(out=img_sb[:, n], in_=src[:, n])
                nc.sync.dma_start(out=w_sb[:, half:], in_=w[:, half:])

                groups = [(n, g) for n in range(PER) for g in range(NG)]
                for cb in range(2):
                    for w0 in range(0, len(groups), wave_sz):
                        wave = groups[w0:w0 + wave_sz]
                        pss = {ng: psum_pool.tile([128, RG * OH], F32,
                                                  name=f"ps{ng}", tag="ps")
                               for ng in wave}
                        if not drop_mm:
                            for t in range(K * K):
                                ki, kj = divmod(t, K)
                                col = (cb * K * K + t) * 128
                                if single_w:
                                    col = 0
                                lhsT = w_sb[:, col: col + 128]
                                for (n, g) in wave:
                                    if flat_rhs:
                                        rhs = img_sb[
                                            :, n].rearrange(
                                            "p h w -> p (h w)")[:, :RG * OH]
                                    else:
                                        rhs = img_sb[
                                            :, n,
                                            g * RG + ki: g * RG + ki + RG,
                                            kj: kj + OH]
                                    if mm_split == 1:
                                        nc.tensor.matmul(
                                            pss[(n, g)], lhsT, rhs,
                                            start=(t == 0),
                                            stop=(t == K * K - 1))
                                    else:
                                        assert flat_rhs
                                        cw = RG * OH // mm_split
                                        for s in range(mm_split):
                                            nc.tensor.matmul(
                                                pss[(n, g)][
                                                    :, s * cw:(s + 1) * cw],
                                                lhsT,
                                                rhs[:, s * cw:(s + 1) * cw],
                                                start=(t == 0),
                                                stop=(t == K * K - 1))
                        if drop_out or drop_mm:
                            continue
                        for j, (n, g) in enumerate(wave):
                            dma_eng = (nc.scalar if out_ring == "scalar"
                                       else nc.sync)
                            dst = out[n, cb * 128:(cb + 1) * 128,
                                      g * RG:(g + 1) * RG]
                            if drain_engine == "psdma":
                                dma_eng.dma_start(
                                    out=dst,
                                    in_=pss[(n, g)].rearrange(
                                        "p (r x) -> p r x", r=RG))
                                continue
                            ob = outp.tile([128, RG * OH], F32,
                                           name="ob", tag="ob")
                            eng = drain_engine
                            if eng == "alt":
                                eng = "vector" if j % 2 == 0 else "scalar"
                            if eng == "vector":
                                nc.vector.tensor_copy(ob, pss[(n, g)])
                            else:
                                nc.scalar.copy(ob, pss[(n, g)])
                            dma_eng.dma_start(
                                out=dst,
                                in_=ob.rearrange("p (r x) -> p r x", r=RG))

            if reps == 1:
                body()
            else:
                with tc.For_i(0, reps):
                    body()

    if DEDUP:
        _dedup_ldweights(nc)
    nc.finalize()
    return nc


def build(reps=1, **kw):
    return _build(reps=reps, **kw)


def _prep(img: np.ndarray, filtro: np.ndarray):
    from ml_dtypes import bfloat16

    img_bf = np.ascontiguousarray(
        np.asarray(img, dtype=np.float32)).astype(bfloat16)
    filtro = np.asarray(filtro, dtype=np.float32)
    # w[ci, ((cb*9 + ki*3+kj)*128 + co128] = filtro[cb*128+co128, ci, ki, kj]
    wt = np.transpose(filtro, (1, 2, 3, 0)).reshape(C_IN, K, K, 2, 128)
    wt = np.ascontiguousarray(np.transpose(wt, (0, 3, 1, 2, 4))).reshape(
        C_IN, 2 * K * K * 128).astype(bfloat16)
    return img_bf, wt


def kernel(img: np.ndarray, filtro: np.ndarray) -> np.ndarray:
    from concourse.bass_utils import run_bass_kernel_spmd

    img_bf, wt = _prep(img, filtro)

    if "nc" not in _CACHE:
        _CACHE["nc"] = _build()
    nc = _CACHE["nc"]

    in_maps = [
        {"img": np.ascontiguousarray(img_bf[c * PER:(c + 1) * PER]),
         "w": wt}
        for c in range(N_CORES)
    ]
    res = run_bass_kernel_spmd(nc, in_maps, list(range(N_CORES)))
    return np.concatenate(
        [res.results[c]["out"] for c in range(N_CORES)], axis=0)


# revision 22
# speedup vs baseline: 1.0420x; 1.0420x over previous
"""3x3 VALID conv (NCHW) on 8 Trainium2 NeuronCores, data-parallel on batch.

Contract: kernel(img, filtro) takes the FULL inputs
  img    [32, 128, 56, 56] f32
  filtro [256, 128, 3, 3]  f32
and returns the FULL output [32, 256, 54, 54] f32.

Strategy (per core, batch shard of 4 images):
- Inputs are host-cast to bf16 (rel-err 2.2e-3 measured, gate 2e-2).
  bf16 matmuls stream 1 col/cycle like fp32r, but their weight loads are
  emitted as standalone Ldweights instructions that the PE's 64-deep
  reorder window hides behind in-flight matmuls - fp32r self-loading
  matmuls instead pay a serial ~107ns reload inside every matmul.
  A post-Tile IR pass (_dedup_ldweights) drops Ldweights that reload the
  stationary already in the array (taps-outer reuses each load 4x).
- img in SBUF channels-on-partitions: [ci=128, n, h, w] bf16 (25KB/part).
- w host-packed to [ci, cb, tap, co128] so each (cb, tap) slice
  [128, 128] is one stationary load.
- Schedule: cb-major; the 24 (image, row-group) tiles per cb are
  processed in waves of 4 (9 rows x 54 = 486 cols = one PSUM bank each),
  taps outer. With 8 PSUM banks, consecutive waves always land on fresh
  banks, so a wave's drains have a full wave (~9us) to finish - no PSUM
  WAR stall and no head-of-line wait on the wave's first Ldweights.
- Drain: DVE copies each bank to an SBUF tile; out DMA goes on the ACT
  HWDGE ring (nc.scalar.dma_start) so stores never head-of-line block
  the SP ring that prefetches the next iteration's images.
- Measured (For_i slope, 8 cores): ~117us/rep steady state vs a
  ~105-115us pure-matmul floor (PE sustains ~2.0GHz under load, P0);
  the fp32r baseline was 147us.
"""
from contextlib import ExitStack

import numpy as np

BATCH, C_IN, C_OUT, H, K = 32, 128, 256, 56, 3
OH = H - K + 1  # 54
N_CORES = 8
PER = BATCH // N_CORES  # 4
RG = 9          # output rows per matmul group; 9*54=486 <= 512-f32 PSUM bank
NG = OH // RG   # 6

_CACHE = {}
DEDUP = True


def _dedup_ldweights(nc):
    """Remove Ldweights whose stationary AP matches the weights already in
    the PE array (tile legalization emits one per matmul; taps-outer reuses
    each load 6x). Any waits/updates on a removed load move to the next PE
    instruction."""
    removed = 0
    for blk in nc.m.functions[0].blocks:
        insts = list(blk.instructions)
        last_w = None
        drop, pending = [], []
        for i, inst in enumerate(insts):
            if str(getattr(inst, "engine", "")) != "EngineType.PE":
                continue
            if pending and inst.opcode in ("Matmult", "Ldweights"):
                si = inst.sync_info
                waits = list(si.on_wait) if si else []
                ups = list(si.on_update) if si else []
                from concourse import mybir as _mb
                for psi in pending:
                    waits += list(psi.on_wait)
                    ups += list(psi.on_update)
                inst.sync_info = _mb.SyncInfo(on_wait=waits, on_update=ups)
                pending = []
            if inst.opcode == "Ldweights":
                sig = str(inst.ins[0])
                if sig == last_w:
                    si = inst.sync_info
                    if si and (len(si.on_wait) or len(si.on_update)):
                        pending.append(si)
                    drop.append(i)
                else:
                    last_w = sig
            elif inst.opcode == "Matmult":
                if inst.ldweights is not False:
                    last_w = str(inst.ins[1])
            else:
                last_w = None
        assert not pending
        for i in reversed(drop):
            del blk.instructions[i]
        removed += len(drop)
    return removed


def _build(reps=1, drop_out=False, drop_mm=False, flat_rhs=False,
           single_w=False, drain_engine="vector", out_ring="scalar",
           wave_sz=4, in_bufs=2, mm_split=1):
    import concourse.tile as tile
    from concourse import bacc, mybir

    BF = mybir.dt.bfloat16
    F32 = mybir.dt.float32
    first_chunks = 3

    nc = bacc.Bacc(None, target_bir_lowering=False)
    img = nc.declare_dram_parameter("img", [PER, C_IN, H, H], BF,
                                    isOutput=False)
    w = nc.declare_dram_parameter("w", [C_IN, 2 * K * K * 128], BF,
                                  isOutput=False)
    out = nc.declare_dram_parameter("out", [PER, C_OUT, OH, OH], F32,
                                    isOutput=True)

    with tile.TileContext(nc) as tc:
        with ExitStack() as ctx:
            wpool = ctx.enter_context(
                tc.tile_pool(name="wpool", bufs=in_bufs))
            imgpool = ctx.enter_context(
                tc.tile_pool(name="imgpool", bufs=in_bufs))
            psum_pool = ctx.enter_context(
                tc.tile_pool(name="psum", bufs=8, space="PSUM"))
            outp = ctx.enter_context(tc.tile_pool(name="outp", bufs=8))

            def body():
                w_sb = wpool.tile([C_IN, 2 * K * K * 128], BF)
                # cb=0 block first so the first wave's weights land early
                half = K * K * 128
                nc.sync.dma_start(out=w_sb[:, :half], in_=w[:, :half])
                img_sb = imgpool.tile([C_IN, PER, H, H], BF)
                src = img.rearrange("n c h w -> c n h w")
                bounds = [0]
                step = (H + first_chunks - 1) // first_chunks
                while bounds[-1] < H:
                    bounds.append(min(bounds[-1] + step, H))
                for r0, r1 in zip(bounds[:-1], bounds[1:]):
                    nc.sync.dma_start(out=img_sb[:, 0, r0:r1],
                                      in_=src[:, 0, r0:r1])
                for n in range(1, PER):
                    nc.sync.dma_start(out=img_sb[:, n], in_=src[:, n])
                nc.sync.dma_start(out=w_sb[:, half:], in_=w[:, half:])

                groups = [(n, g) for n in range(PER) for g in range(NG)]
                for cb in range(2):
                    for w0 in range(0, len(groups), wave_sz):
                        wave = groups[w0:w0 + wave_sz]
                        pss = {ng: psum_pool.tile([128, RG * OH], F32,
                                                  name=f"ps{ng}", tag="ps")
                               for ng in wave}
                        if not drop_mm:
                            for t in range(K * K):
                                ki, kj = divmod(t, K)
                                col = (cb * K * K + t) * 128
                                if single_w:
                                    col = 0
                                lhsT = w_sb[:, col: col + 128]
                                for (n, g) in wave:
                                    if flat_rhs:
                                        rhs = img_sb[
                                            :, n].rearrange(
                                            "p h w -> p (h w)")[:, :RG * OH]
                                    else:
                                        rhs = img_sb[
                                            :, n,
                                            g * RG + ki: g * RG + ki + RG,
                                            kj: kj + OH]
                                    if mm_split == 1:
                                        nc.tensor.matmul(
                                            pss[(n, g)], lhsT, rhs,
                                            start=(t == 0),
                                            stop=(t == K * K - 1))
                                    else:
                                        assert flat_rhs
                                        cw = RG * OH // mm_split
                                        for s in range(mm_split):
                                            nc.tensor.matmul(
                                                pss[(n, g)][
                                                    :, s * cw:(s + 1) * cw],
                                                lhsT,
                                                rhs[:, s * cw:(s + 1) * cw],
                                                start=(t == 0),
                                                stop=(t == K * K - 1))
                        if drop_out or drop_mm:
                            continue
                        for j, (n, g) in enumerate(wave):
                            dma_eng = (nc.scalar if out_ring == "scalar"
                                       else nc.sync)
                            dst = out[n, cb * 128:(cb + 1) * 128,
                                      g * RG:(g + 1) * RG]
                            if drain_engine == "psdma":
                                dma_eng.dma_start(
                                    out=dst,
                                    in_=pss[(n, g)].rearrange(
                                        "p (r x) -> p r x", r=RG))
                                continue
                            ob = outp.tile([128, RG * OH], F32,
                                           name="ob", tag="ob")
                            eng = drain_engine
                            if eng == "alt":
                                eng = "vector" if j % 2 == 0 else "scalar"
                            if eng == "vector":
                                nc.vector.tensor_copy(ob, pss[(n, g)])
                            else:
                                nc.scalar.copy(ob, pss[(n, g)])
                            dma_eng.dma_start(
                                out=dst,
                                in_=ob.rearrange("p (r x) -> p r x", r=RG))

            if reps == 1:
                body()
            else:
                with tc.For_i(0, reps):
                    body()

    if DEDUP:
        _dedup_ldweights(nc)
    nc.finalize()
    return nc


def build(reps=1, **kw):
    return _build(reps=reps, **kw)


def _prep(img: np.ndarray, filtro: np.ndarray):
    from ml_dtypes import bfloat16

    img_bf = np.ascontiguousarray(
        np.asarray(img, dtype=np.float32)).astype(bfloat16)
    filtro = np.asarray(filtro, dtype=np.float32)
    # w[ci, ((cb*9 + ki*3+kj)*128 + co128] = filtro[cb*128+co128, ci, ki, kj]
    wt = np.transpose(filtro, (1, 2, 3, 0)).reshape(C_IN, K, K, 2, 128)
    wt = np.ascontiguousarray(np.transpose(wt, (0, 3, 1, 2, 4))).reshape(
        C_IN, 2 * K * K * 128).astype(bfloat16)
    return img_bf, wt


def kernel(img: np.ndarray, filtro: np.ndarray) -> np.ndarray:
    from concourse.bass_utils import run_bass_kernel_spmd

    img_bf, wt = _prep(img, filtro)

    if "nc" not in _CACHE:
        _CACHE["nc"] = _build()
    nc = _CACHE["nc"]

    in_maps = [
        {"img": np.ascontiguousarray(img_bf[c * PER:(c + 1) * PER]),
         "w": wt}
        for c in range(N_CORES)
    ]
    res = run_bass_kernel_spmd(nc, in_maps, list(range(N_CORES)))
    return np.concatenate(
        [res.results[c]["out"] for c in range(N_CORES)], axis=0)
